# revision 8
# baseline (speedup 1.0000x reference)
"""Trainium2 Bass kernel for partial-channel binary dropout with sum compensation.

Computes, for selected channels idx (len K) of X[..., F]:
    sub    = X[..., idx]
    masked = sub * mask                     (mask==1 -> dropped)
    comp   = sum(masked, -1) / K
    out[..., idx] = sub - masked + comp     (zero dropped, redistribute mass)
    out elsewhere = X

Strategy: flatten X to rows (B*C*T, F); shard rows over 8 NeuronCores
(data-parallel, no cross-core communication). Per core, rows are blocked
per-partition (partition p owns a contiguous row range) so every DMA moves
large contiguous per-partition chunks. Per row, one DVE scalar_tensor_tensor
computes masked values + their row-sum in a single pass (accum_out); the
scalar engine scales the sums by 1/K; a second in-place scalar_tensor_tensor
writes the compensated values back into the X tile, which is then stored as
full contiguous rows.
"""

import numpy as np

B, C, T, F, K = 32, 16, 512, 256, 128
N_CORES = 8
R_TOTAL = B * C * T                 # 262144 rows
R_CORE = R_TOTAL // N_CORES         # 32768 rows per core
P = 128                             # SBUF partitions
RPP = R_CORE // P                   # 256 rows per partition
CHUNK = 16                          # rows per partition per tile
N_TILES = RPP // CHUNK
INV_K = 1.0 / K

TRACE = False                       # set by test harness for profiling
LAST_EXEC_NS = None
LAST_RESULTS = None

_nc_cache = {}


def _install_ntff_hook_shim():
    """Provide antenv.axon_hooks (missing from this image) so that
    run_bass_kernel_spmd(trace=True) can drive NTFF capture through the
    axon .so — mirrors trn_agent_boot/trn_boot.py's ctypes path."""
    import sys
    import types
    import ctypes
    import contextlib

    try:
        from antenv.axon_hooks import get_axon_ntff_profile_hook  # noqa: F401
        return  # real module present
    except ImportError:
        pass

    so_path = "/opt/axon/libaxon_pjrt.so"
    lib = ctypes.CDLL(so_path)
    if not hasattr(lib, "axon_start_nrt_profile"):
        return
    lib.axon_start_nrt_profile.argtypes = [
        ctypes.POINTER(ctypes.c_int64),
        ctypes.c_size_t,
    ]
    lib.axon_start_nrt_profile.restype = ctypes.c_int64
    lib.axon_stop_nrt_profile.argtypes = [ctypes.c_char_p]
    lib.axon_stop_nrt_profile.restype = ctypes.c_int64

    @contextlib.contextmanager
    def _hook(output_dir, device_ids):
        import jax

        jax.devices()
        if device_ids:
            ids = (ctypes.c_int64 * len(device_ids))(*device_ids)
            rc = lib.axon_start_nrt_profile(ids, len(device_ids))
        else:
            rc = lib.axon_start_nrt_profile(None, 0)
        if rc != 0:
            raise RuntimeError(f"axon_start_nrt_profile rc={rc}")
        try:
            yield
        finally:
            n = lib.axon_stop_nrt_profile(str(output_dir).encode())
            print(f"ntff profile: {n} file(s) written to {output_dir}")

    mod = types.ModuleType("antenv.axon_hooks")
    mod.get_axon_ntff_profile_hook = lambda: _hook
    mod.set_axon_ntff_profile_hook = lambda h: None
    sys.modules["antenv.axon_hooks"] = mod


def _build_bass(idx_off: int, idx_step: int):
    import concourse.bacc as bacc
    import concourse.mybir as mybir
    from concourse.tile import TileContext

    # Bacc (not raw Bass): its compile() pass splits multi-sem sync waits,
    # which TRN2 instruction encodings can't carry (max 1 wait/instruction)
    nc = bacc.Bacc()
    x = nc.dram_tensor("x", (R_CORE, F), mybir.dt.float32, kind="ExternalInput")
    m = nc.dram_tensor("m", (R_CORE, K), mybir.dt.uint8, kind="ExternalInput")
    y = nc.dram_tensor("y", (R_CORE, F), mybir.dt.float32, kind="ExternalOutput")

    xr = x[:].rearrange("(p n) f -> p n f", p=P)
    mr = m[:].rearrange("(p n) k -> p n k", p=P)
    yr = y[:].rearrange("(p n) f -> p n f", p=P)

    lo = idx_off
    hi = idx_off + idx_step * K

    with TileContext(nc) as tc:
        with (
            tc.tile_pool(name="xp", bufs=3) as xp,
            tc.tile_pool(name="mp", bufs=3) as mp,
            tc.tile_pool(name="wp", bufs=3) as wp,
            tc.tile_pool(name="sp", bufs=3) as sp,
        ):
            for j in range(N_TILES):
                xt = xp.tile([P, CHUNK, F], mybir.dt.float32, name="xt")
                mt = mp.tile([P, CHUNK, K], mybir.dt.float32, name="mt")
                nc.sync.dma_start(out=xt, in_=xr[:, j * CHUNK:(j + 1) * CHUNK, :])
                # SWDGE casts uint8 -> f32 in the DMA datapath
                nc.gpsimd.dma_start(out=mt, in_=mr[:, j * CHUNK:(j + 1) * CHUNK, :])
                wt = wp.tile([P, CHUNK, K], mybir.dt.float32, name="wt")
                ms = sp.tile([P, CHUNK], mybir.dt.float32, name="ms")
                cs = sp.tile([P, CHUNK], mybir.dt.float32, name="cs")
                for r in range(CHUNK):
                    xe = xt[:, r, lo:hi:idx_step]
                    nc.vector.scalar_tensor_tensor(
                        out=wt[:, r, :], in0=xe, scalar=0.0, in1=mt[:, r, :],
                        op0=mybir.AluOpType.add, op1=mybir.AluOpType.mult,
                        accum_out=ms[:, r:r + 1],
                    )
                nc.vector.tensor_scalar_mul(cs[:, :], ms[:, :], INV_K)
                for r in range(CHUNK):
                    xe = xt[:, r, lo:hi:idx_step]
                    nc.vector.scalar_tensor_tensor(
                        out=xe, in0=xe, scalar=cs[:, r:r + 1], in1=wt[:, r, :],
                        op0=mybir.AluOpType.add, op1=mybir.AluOpType.subtract,
                    )
                nc.sync.dma_start(out=yr[:, j * CHUNK:(j + 1) * CHUNK, :], in_=xt)
    nc.finalize()
    return nc


def _numpy_fallback(X, idx, mask):
    # exact emulation of the reference for non-affine idx (never hit with the
    # shipped setup_inputs, which uses idx = 2*arange(K))
    sub = X[..., idx]
    power = sub.sum(-1)
    zeroed = np.where(mask, np.float32(0), sub)
    comp = ((power - zeroed.sum(-1)) / np.float32(K)).astype(np.float32)
    new_sub = zeroed + comp[..., None]
    out = X.copy()
    out[..., idx] = new_sub
    return out


def kernel(X, idx, mask):
    global LAST_EXEC_NS, LAST_RESULTS
    X = np.asarray(X, dtype=np.float32)
    idx = np.asarray(idx, dtype=np.int32)
    mask = np.asarray(mask)

    assert X.shape == (B, C, T, F) and idx.shape == (K,) and mask.shape == (B, C, T, K)

    # the kernel bakes the (necessarily affine) gather pattern into its APs
    off = int(idx[0])
    step = int(idx[1] - idx[0]) if K > 1 else 1
    affine = (
        K > 1
        and step > 0
        and bool(np.all(np.diff(idx.astype(np.int64)) == step))
        and 0 <= off
        and off + step * (K - 1) < F
    )
    if not affine:
        return _numpy_fallback(X, idx, mask.astype(bool))

    from concourse.bass_utils import run_bass_kernel_spmd

    key = (off, step)
    if key not in _nc_cache:
        _nc_cache[key] = _build_bass(off, step)
    nc = _nc_cache[key]

    Xf = np.ascontiguousarray(X.reshape(R_TOTAL, F))
    Mf = np.ascontiguousarray(mask.reshape(R_TOTAL, K)).view(np.uint8)

    in_maps = [
        {
            "x": Xf[c * R_CORE:(c + 1) * R_CORE],
            "m": Mf[c * R_CORE:(c + 1) * R_CORE],
        }
        for c in range(N_CORES)
    ]

    kw = {}
    if TRACE:
        _install_ntff_hook_shim()
        kw = dict(trace=True, trace_cores=[0])
    res = run_bass_kernel_spmd(nc, in_maps, core_ids=list(range(N_CORES)), **kw)
    LAST_EXEC_NS = res.exec_time_ns
    LAST_RESULTS = res

    out = np.concatenate([r["y"] for r in res.results], axis=0)
    return out.reshape(B, C, T, F)
